# revision 54
# baseline (speedup 1.0000x reference)
"""AsymFormer forward on 8 TRN2 NeuronCores — data-parallel over batch.

v4 design (from v3):
 - B=8 -> one batch element per core, no collectives.
 - Phase 1 (relation branch): same matmul structure, but the rsqrt
   Newton-refinement chain (6 gpsimd ops/group) is reduced to a single
   pow(x,-0.5) like the LN path; the var+eps PSUM->SBUF move runs
   entirely on ACT (keeps DVE a pure back-to-back p1_M stream) and is
   batched per PAIR of groups (adjacent var slots in vbig), as is the
   pow — the phase-1 cadence is max(ACT 612+106, DVE 658) per group.
 - Phase 2 attention restructured:
   * per-(head,kt) score groups ordered R-add first (depends only on
     R_T), then the q.k matmul accumulates on top.
   * o = E@v computed q-major (lhsT = E chunk): all 8 heads + softmax
     denominators land in one PSUM bank per q-half -> one reciprocal +
     one broadcast-multiply per half per depth (replaces 8 reciprocal +
     8 partition_broadcast + 8 multiplies).
   * proj applied with ogT as lhsT -> output lands token-major, residual
     add needs no transpose; v computed k-major directly (no vT
     transposes, no v-bias ACT op; generic-bias fallbacks ride blobF).
 - Depth-0 LN1/qkv is NOT hoisted ahead of phase 1 (PRE0=0): its ACT
   ops would inject into the cadence-critical phase-1 ACT stream. Its
   LN transposes still route via psmo (early=True) because the vbig
   bank carries phase-1 var traffic until just before depth 0.
 - MLP gelu: x2 squared straight from PSUM (parallel with xg); t1g on
   ACT right after x2 (no cross-engine hop); the final "x + x*tanh" add
   is absorbed into the h2 matmul (PE accumulates xg@W + xt2@W); all
   tensor_tensor hops on DVE (GPSIMD's higher per-op latency cost more
   in chain latency than it saved in DVE occupancy).
 - Hardware constraints learned: DMAs only from SP/ACT (HWDGE) queues;
   TensorScalar is NOT a valid Pool opcode; PSUM accumulation groups
   must close before another group's start=True touches the same bank.
"""

import os
import sys

sys.path.insert(0, "/opt/trn_rl_repo")

import numpy as np

import concourse.bacc as bacc
import concourse.bass as bass
import concourse.mybir as mybir
import concourse.tile as tile
from concourse.bass_utils import run_bass_kernel_spmd

B, N, C, H, DP = 8, 256, 128, 8, 4
HS = C // H  # 16
SCALE = 0.25
NN = N * N
F32 = mybir.dt.float32
BF16 = mybir.dt.bfloat16
F32R = mybir.dt.float32r
NPBF = mybir.dt.np(BF16)
AF = mybir.ActivationFunctionType
ALU = mybir.AluOpType
GELC0 = 0.7978845608028654
GELC1 = 0.044715

NG = 32          # phase-1 groups (2048 rel-rows each)
# blobA: qkw | vw | mw1 | mw2h | connS2 | Wd
BLOBA = DP * 2 * 2 * C + DP * C + DP * C + DP * C + 2 * 2 * N + 90
# blobB layout (f32, 128 partitions): qkb | vb | projb | mb1 | mb2 | bj | idf
BLOBB = DP * 2 * 2 + DP + DP + DP + DP + 1 + 128
# blobF: projW | projbB | vbB | mb2B  (128 partitions, bf16)
BLOBF = 4 * DP * C
DP_EMIT = int(os.environ.get("DP_EMIT", DP))
SAFE_SPK = os.environ.get("SAFE_SPK", "1") == "1"   # per-j2 score groups
ACT_COPY = os.environ.get("ACT_COPY", "0") == "1"   # ACT does half the PSUM->SBUF copies
PRE0 = os.environ.get("PRE0", "0") == "1"           # hoist depth-0 qkv ahead of phase 1

last_results = None


def _fold(inp):
    f = lambda k: np.asarray(inp[k], np.float32)
    w = {}
    # relation encoder collapse 26->128
    Wc = f("re_w1") @ f("re_w2") @ f("re_w3")
    bc = (f("re_b1") @ f("re_w2") + f("re_b2")) @ f("re_w3") + f("re_b3")
    P = np.eye(128, dtype=np.float64) - 1.0 / 128.0
    Mh = np.concatenate(
        [P @ Wc.T.astype(np.float64), P @ bc.astype(np.float64).reshape(128, 1)],
        axis=1,
    )
    G = Mh.T @ Mh
    Rc = np.linalg.cholesky(G + 1e-14 * np.eye(27)).T  # upper, Rc.T@Rc = G
    Rc = Rc.astype(np.float32)
    # block-diag yc matmul: (128, 128), [32c+f, 32c+r] = Rc[r, f]
    RcBD = np.zeros((128, 128), np.float32)
    for g in range(4):
        RcBD[32 * g : 32 * g + 27, 32 * g : 32 * g + 27] = Rc.T
    w["RcBD"] = RcBD.astype(NPBF)
    # o27sel (128, 4): [32g+r, g] = 1/128 for r < 27
    o27 = np.zeros((128, 4), np.float32)
    for g in range(4):
        o27[32 * g : 32 * g + 27, g] = 1.0 / 128.0
    w["o27sel"] = o27.astype(NPBF)
    # ln2+SCALE fold into rconv -> Wr2 (128, 32), mean-centered
    Wr = np.empty((128, DP, H), np.float32)
    br = np.empty((DP, H), np.float32)
    for i in range(DP):
        Wr[:, i, :] = SCALE * (f("ln2_g")[i][:, None] * f("rconv_w")[i])
        br[i] = SCALE * (f("ln2_b")[i] @ f("rconv_w")[i] + f("rconv_b")[i])
    Wr2 = Wr.reshape(128, DP * H)
    Wr2 = Wr2 - np.ones((128, 1), np.float32) * (Wr2.sum(0, keepdims=True) / 128.0)
    WqA = np.concatenate([Wc @ Wr2, (Wr2.T @ bc).reshape(1, 32)], axis=0)  # (27,32)
    WqBD = np.zeros((128, 128), np.float32)
    for c in range(4):
        WqBD[32 * c : 32 * c + 27, 32 * c : 32 * c + 32] = WqA
    w["WqA"] = WqBD.astype(NPBF)
    w["brO"] = np.ascontiguousarray(
        np.broadcast_to(br.reshape(-1), (128, 16, 32)).reshape(128, 512)
    ).astype(NPBF)
    # joint encoder collapse 96->128
    Wj = f("je_w1") @ f("je_w2") @ f("je_w3")
    bj = (f("je_b1") @ f("je_w2") + f("je_b2")) @ f("je_w3") + f("je_b3")
    w["Wj"] = np.ascontiguousarray(Wj)
    w["bj"] = np.ascontiguousarray(bj.reshape(128, 1))
    # per-depth: ln1 into qkv (+SCALE on q), ln3 into mw1
    qkvw = np.empty((DP, C, 3 * C), np.float32)
    qkvb = np.empty((DP, 3 * C), np.float32)
    mw1 = np.empty((DP, C, C), np.float32)
    mb1 = np.empty((DP, C), np.float32)
    for i in range(DP):
        qkvw[i] = f("ln1_g")[i][:, None] * f("qkv_w")[i]
        qkvb[i] = f("ln1_b")[i] @ f("qkv_w")[i] + f("qkv_b")[i]
        qkvw[i][:, :C] *= SCALE
        qkvb[i][:C] *= SCALE
        mw1[i] = f("ln3_g")[i][:, None] * f("mw1")[i]
        mb1[i] = f("ln3_b")[i] @ f("mw1")[i] + f("mb1")[i]
    # qk: heads 4u+j at 32j+s (s<16) within tile u
    qkw = np.zeros((C, DP, 2, 2, C), np.float32)
    qkb = np.zeros((C, DP, 2, 2), np.float32)
    for i in range(DP):
        for t in range(2):
            wt = qkvw[i][:, t * C : (t + 1) * C]
            bt = qkvb[i][t * C : (t + 1) * C]
            for h in range(H):
                u, j = divmod(h, 4)
                qkw[:, i, t, u, 32 * j : 32 * j + HS] = wt[:, h * HS : (h + 1) * HS]
                qkb[32 * j : 32 * j + HS, i, t, u] = bt[h * HS : (h + 1) * HS]
    w["qkw"] = qkw.astype(NPBF)
    w["qkb"] = np.ascontiguousarray(qkb)
    w["vw"] = np.ascontiguousarray(qkvw.transpose(1, 0, 2)[:, :, 2 * C :]).astype(NPBF)
    vb = qkvb[:, 2 * C :]  # (DP, C)
    w["vb"] = np.ascontiguousarray(vb.T)
    # proj token-major: rows are (h*HS+s) = proj_w rows directly
    w["projW"] = np.ascontiguousarray(
        f("proj_w").transpose(1, 0, 2)
    ).astype(NPBF)  # (C, DP, C)
    projb = f("proj_b")  # (DP, C)
    w["projb"] = np.ascontiguousarray(projb.T)
    w["mw1"] = np.ascontiguousarray(mw1.transpose(1, 0, 2)).astype(NPBF)
    w["mb1"] = np.ascontiguousarray(mb1.T)
    w["mw2h"] = np.ascontiguousarray(0.5 * f("mw2").transpose(1, 0, 2)).astype(NPBF)
    w["mb2"] = np.ascontiguousarray(f("mb2").T)
    # decoder with final LN affine folded
    Wdc = f("dw1") @ f("dw2") @ f("dw3")
    Wd = f("ng")[:, None] * Wdc
    bd = f("nb") @ Wdc + (f("db1") @ f("dw2") + f("db2")) @ f("dw3") + f("db3")
    w["Wd"] = np.ascontiguousarray(Wd).astype(NPBF)
    w["bd"] = np.ascontiguousarray(bd.reshape(90, 1))
    idb = np.eye(128, dtype=np.float32).astype(NPBF)
    idf = np.eye(128, dtype=np.float32)
    out = {}
    out["_Apre"] = np.concatenate([
        w["qkw"].reshape(128, -1), w["vw"].reshape(128, -1),
        w["mw1"].reshape(128, -1), w["mw2h"].reshape(128, -1),
    ], axis=1)
    out["_Apost"] = w["Wd"]
    out["blobI"] = np.concatenate([w["o27sel"], idb, w["brO"]], axis=1)
    out["blobB"] = np.concatenate([
        w["qkb"].reshape(128, -1), w["vb"], w["projb"], w["mb1"], w["mb2"],
        w["bj"], idf,
    ], axis=1).astype(np.float32)
    out["_Wj"] = w["Wj"]
    out["blobD"] = w["RcBD"]
    out["blobE"] = w["WqA"]
    # blobF: projW | projbB | vbB | mb2B (all broadcast along partitions
    # for the bias tiles; only read when the corresponding bias is nonzero)
    projbB = np.broadcast_to(projb.reshape(1, DP * C), (128, DP * C))
    vbB = np.broadcast_to(vb.reshape(1, DP * C), (128, DP * C))
    mb2B = np.broadcast_to(f("mb2").reshape(1, DP * C), (128, DP * C))
    out["blobF"] = np.ascontiguousarray(np.concatenate([
        w["projW"].reshape(128, -1).astype(np.float32),
        projbB, vbB, mb2B,
    ], axis=1)).astype(NPBF)
    out["blobH"] = w["bd"]
    out["_brz"] = bool(np.all(w["brO"] == 0))
    out["_m2z"] = bool(np.all(w["mb2"] == 0))
    out["_pbz"] = bool(np.all(projb == 0))
    out["_vbz"] = bool(np.all(vb == 0))
    return out


def _build(br_zero=True, m2z=True, pbz=True, vbz=True):
    nc = bacc.Bacc(None, target_bir_lowering=False)

    def din(name, shape, dt=F32):
        return nc.dram_tensor(name, list(shape), dt, kind="ExternalInput")

    relq_d = din("relq", (8, 128, 2048), BF16)
    # const blobs (concatenated along free dim, per partition-count/dtype)
    blobA_d = din("blobA", (128, BLOBA), BF16)   # 128-part bf16 weights
    blobB_d = din("blobB", (128, BLOBB))         # 128-part f32 biases/idf
    blobC_d = din("blobC", (96, 128 + N), F32R)  # Wj | jT
    blobD_d = din("blobD", (128, 128), BF16)     # RcBD
    blobI_d = din("blobI", (128, 132 + 512), BF16)  # o27sel | idb | brO
    blobE_d = din("blobE", (128, 128), BF16)     # WqBD block-diag
    blobF_d = din("blobF", (128, BLOBF), BF16)   # projW | projbB | vbB | mb2B
    blobH_d = din("blobH", (90, 1))              # bd
    out_d = nc.dram_tensor("out", [N, 90], F32, kind="ExternalOutput")

    from contextlib import ExitStack

    with tile.TileContext(nc) as tc, ExitStack() as ctx, nc.allow_low_precision(
        reason="bf16 pipeline; end-to-end precision checked in test"
    ):
        const = ctx.enter_context(tc.tile_pool(name="const", bufs=1))
        zin = ctx.enter_context(tc.tile_pool(name="zin", bufs=8))
        st = ctx.enter_context(tc.tile_pool(name="st", bufs=8))
        wrk = ctx.enter_context(tc.tile_pool(name="wrk", bufs=4))
        wrk4 = ctx.enter_context(tc.tile_pool(name="wrk4", bufs=4))
        ekp = ctx.enter_context(tc.tile_pool(name="ekp", bufs=6))
        psm = ctx.enter_context(tc.tile_pool(name="psm", bufs=int(os.environ.get("PSMB","3")), space="PSUM"))
        psma = ctx.enter_context(tc.tile_pool(name="psma", bufs=2, space="PSUM"))
        psmv = ctx.enter_context(tc.tile_pool(name="psmv", bufs=1, space="PSUM"))
        psmo = ctx.enter_context(tc.tile_pool(name="psmo", bufs=int(os.environ.get("PSMOB","2")), space="PSUM"))

        def cload(dt_handle, shape, tag, dt=F32, eng=None):
            t = const.tile(list(shape), dt, tag=tag)
            (eng or nc.scalar).dma_start(out=t, in_=dt_handle[:])
            return t

        RcBD_s = cload(blobD_d, (128, 128), "RcBD", BF16)
        WqA_s = cload(blobE_d, (128, 128), "WqA", BF16)
        mhB_s = const.tile([128, 4, 4], F32, tag="mhB")
        nc.vector.memset(mhB_s[:], -0.5)
        eps_s = const.tile([128, 1], F32, tag="eps")
        nc.vector.memset(eps_s[:], 1e-5)
        mh1_s = const.tile([128, 1], F32, tag="mh1")
        nc.vector.memset(mh1_s[:], -0.5)
        mh2_s = const.tile([128, 2, 1], F32, tag="mh2")
        nc.vector.memset(mh2_s[:], -0.5)
        mhP_s = const.tile([128, 2, 16], F32, tag="mhP")
        nc.vector.memset(mhP_s[:], -0.5)
        blobI_s = cload(blobI_d, (128, 132 + 512), "blobI", BF16)
        o27_s = blobI_s[:, 0:4]
        idb_s = blobI_s[:, 4:132]
        brO_s = blobI_s[:, 132:644].rearrange("p (kt q h) -> p kt q h", kt=2, q=8)
        blobC_s = cload(blobC_d, (96, 128 + N), "blobC", F32R)
        Wj_s = blobC_s[:, 0:128]
        jT_s = blobC_s[:, 128 : 128 + N]
        blobB_s = cload(blobB_d, (128, BLOBB), "blobB")
        blobA_s = const.tile([128, BLOBA], BF16, tag="blobA")
        _o = [0]

        def _cutA(n, shape=None):
            v = blobA_s[:, _o[0] : _o[0] + n]
            _o[0] += n
            return v

        qkw_s = _cutA(DP * 2 * 2 * C).rearrange(
            "p (i t u c) -> p i t u c", i=DP, t=2, u=2
        )
        vw_s = _cutA(DP * C).rearrange("p (i c) -> p i c", i=DP)
        mw1_s = _cutA(DP * C).rearrange("p (i c) -> p i c", i=DP)
        mw2h_s = _cutA(DP * C).rearrange("p (i c) -> p i c", i=DP)
        connS2_s = _cutA(2 * 2 * N).rearrange("p (a kt q) -> p a kt q", a=2, kt=2)
        Wd_s = _cutA(90)
        _ob = [0]

        def _cutB(n):
            v = blobB_s[:, _ob[0] : _ob[0] + n]
            _ob[0] += n
            return v

        qkb_s = _cutB(DP * 2 * 2).rearrange("p (i t u) -> p i t u", i=DP, t=2)
        vb_s = _cutB(DP)
        projb_s = _cutB(DP)
        mb1_s = _cutB(DP)
        mb2_s = _cutB(DP)
        bj_s = _cutB(1)
        idf_s = _cutB(128)

        R_T = const.tile([128, 2, N, 32], BF16, tag="R_T")
        vbig = psmv.tile([128, 512], F32, tag="vt")  # shared bank: var slots + transpose slots
        tbig = vbig[:, 128:512].bitcast(BF16)  # (128, 768) bf16 -> 6 transpose slots
        tctr = [0]

        def tslot():
            r = tctr[0] % 6
            tctr[0] += 1
            return tbig[:, 128 * r : 128 * (r + 1)]

        # v_ext tiles (128, H, 18): cols 0-15 = v (per depth), col 16 = ones
        # (softmax denominator rides the same matmul), col 17 = pad
        vxt = []
        for kt in range(2):
            t = const.tile([128, H, 18], BF16, tag=f"vx{kt}")
            nc.vector.memset(t[:, :, 16:18], 0.0)
            nc.vector.memset(t[:, :, 16:17], 1.0)
            vxt.append(t)

        # ---------------- Phase 1: relation branch -> R_T ----------------
        def p1_Y(G, j, relq):
            yc_ps = psm.tile([128, 512], F32, tag="b")
            nc.tensor.matmul(
                yc_ps, RcBD_s[:], relq[:, j * 512 : (j + 1) * 512],
                start=True, stop=True,
            )
            ycsq = wrk.tile([128, 512], BF16, tag="ycsq")
            nc.scalar.activation(ycsq, yc_ps, AF.Square)
            var_q = vbig[:, 16 * (G % 8) : 16 * (G % 8) + 16].rearrange(
                "p (t c) -> p t c", t=4
            )
            for s in range(4):
                nc.tensor.matmul(
                    var_q[:, s, :],
                    ycsq[:, s * 128 : (s + 1) * 128],
                    o27_s[:], start=True, stop=True,
                )
            # veps/pow are emitted per PAIR of groups in p1_VP below

        def p1_VP(G):
            """veps + pow for groups (G-1, G), G odd: their var slots are
            adjacent in vbig, so one ACT op (PSUM->SBUF + eps) and one Pool
            pow cover both — halves the per-group ACT/Pool tax on the
            phase-1 cadence (ACT is the cadence-critical engine)."""
            s0 = 16 * ((G - 1) % 8)
            vpair = vbig[:, s0 : s0 + 32]
            veps = st.tile([128, 2, 16], F32, tag="veps")
            nc.scalar.activation(veps, vpair.rearrange(
                "p (g a) -> p g a", g=2), AF.Identity, bias=eps_s[:])
            rsg = st.tile([128, 2, 16, 1], F32, tag="rsg")
            nc.gpsimd.tensor_tensor(
                out=rsg[:, :, :, 0], in0=veps, in1=mhP_s, op=ALU.pow,
            )
            return (
                rsg[:, 0].rearrange("p (t cc kt) one -> p t cc kt one",
                                    t=4, cc=2),
                rsg[:, 1].rearrange("p (t cc kt) one -> p t cc kt one",
                                    t=4, cc=2),
            )

        def p1_A2(j, relq):
            a_ps = psma.tile([128, 4, 4, 32], F32, tag="a")
            for t in range(4):
                t2 = 4 * j + t
                nc.tensor.matmul(
                    a_ps[:, t, :, :],
                    relq[:, t2 * 128 : (t2 + 1) * 128],
                    WqA_s[:], start=True, stop=True,
                )
            return a_ps

        def p1_M(G, a_ps, rsg):
            out = R_T[:, :, 8 * G : 8 * G + 8, :].rearrange(
                "p kt (t cc) h -> p t cc kt h", t=4
            )
            av = a_ps.rearrange("p t (cc kt) h -> p t cc kt h", cc=2)
            ia, ib = bass.broadcast_tensor_aps(av, rsg)
            nc.vector.tensor_tensor(out=out, in0=ia, in1=ib, op=ALU.mult)
            if not br_zero:
                sl = R_T[:, :, 8 * G : 8 * G + 8, :]
                eng = nc.vector if G % 2 == 0 else nc.gpsimd
                eng.tensor_tensor(out=sl, in0=sl, in1=brO_s[:], op=ALU.add)

        work = []  # (G, relq, j)
        relq_split = os.environ.get("RELQ_SPLIT", "0") == "1"
        for D in range(8):
            relq = zin.tile([128, 2048], BF16, tag="relq")
            eng = nc.scalar if (relq_split and D % 2 == 1) else nc.sync
            eng.dma_start(out=relq, in_=relq_d[D])
            for j in range(4):
                work.append((4 * D + j, relq, j))
        NGW = len(work)
        # big phase-2 blob: issued on the sync queue (after the rel DMAs) so
        # the ACT queue is never blocked behind a long transfer
        nc.sync.dma_start(out=blobA_s, in_=blobA_d[:])
        blobF_s = const.tile([128, BLOBF], BF16, tag="blobF")
        nc.sync.dma_start(out=blobF_s, in_=blobF_d[:])
        projW_s = blobF_s[:, 0 : DP * C].rearrange("p (i c) -> p i c", i=DP)
        projbB_s = blobF_s[:, DP * C : 2 * DP * C].rearrange(
            "p (i c) -> p i c", i=DP
        )
        vbB_s = blobF_s[:, 2 * DP * C : 3 * DP * C].rearrange(
            "p (i h s) -> p i h s", i=DP, h=H
        )
        mb2B_s = blobF_s[:, 3 * DP * C : 4 * DP * C].rearrange(
            "p (i c) -> p i c", i=DP
        )
        bd_s = const.tile([90, 1], F32, tag="bd")
        nc.sync.dma_start(out=bd_s, in_=blobH_d[:])

        # ---------------- joint encoder -> jf (token-major) --------------
        # jf lives in ONE tile (128, 2, 128): both token-halves share a
        # single bn_stats / veps / pow per LayerNorm (bn_stats computes
        # per-segment stats along the middle dim)
        jf_all = const.tile([128, 2, 128], F32, tag="jfA")
        jf = [jf_all[:, 0, :], jf_all[:, 1, :]]
        jp = psmo.tile([128, N], F32, tag="o")
        nc.tensor.matmul(jp, Wj_s[:], jT_s[:], start=True, stop=True)
        jfT = wrk.tile([128, N], F32, tag="jfT")
        nc.scalar.activation(jfT, jp, AF.Identity, bias=bj_s[:])
        for qt in range(2):
            tp = psmo.tile([128, 128], F32, tag="o")
            nc.tensor.transpose(tp, jfT[:, qt * 128 : (qt + 1) * 128], idf_s[:])
            nc.vector.tensor_copy(out=jf[qt], in_=tp[:])

        def layer_norm_t(tag, early=False):
            """token-major standardize -> feature-major (128, 256) bf16.

            early=True routes the transpose through psmo (phase 1 owns the
            vbig bank that tslot() lives in — a PE write there while phase-1
            DVE/ACT read var slots would be a PSUM bank collision).
            """
            xT = wrk.tile([128, N], BF16, tag=f"xT{tag}")
            for qt in range(2):
                st6 = st.tile([128, 6], F32, tag="st6")
                nc.vector.bn_stats(out=st6, in_=jf[qt])
                mv = st.tile([128, 2], F32, tag="mv")
                nc.vector.bn_aggr(out=mv, in_=st6[:])
                veps = st.tile([128, 1], F32, tag="veps1")
                nc.vector.tensor_scalar(
                    out=veps, in0=mv[:, 1:2], scalar1=1e-5, scalar2=None,
                    op0=ALU.add,
                )
                rs0 = st.tile([128, 1], F32, tag="rs0")
                # gpsimd supports tensor_tensor only (TensorScalar fails the
                # hardware engine check on Pool)
                nc.gpsimd.tensor_tensor(
                    out=rs0, in0=veps, in1=mh1_s[:], op=ALU.pow
                )
                xh = wrk4.tile([128, 128], BF16, tag="xh")
                nc.vector.tensor_scalar(
                    out=xh, in0=jf[qt], scalar1=mv[:, 0:1], scalar2=rs0[:],
                    op0=ALU.subtract, op1=ALU.mult,
                )
                if early:
                    tp = psmo.tile([128, 128], BF16, tag="o")
                else:
                    tp = tslot()
                nc.tensor.transpose(tp, xh[:], idb_s[:])
                if qt == 0 or not ACT_COPY:
                    nc.vector.tensor_copy(
                        out=xT[:, qt * 128 : (qt + 1) * 128], in_=tp
                    )
                else:
                    nc.scalar.activation(
                        xT[:, qt * 128 : (qt + 1) * 128], tp, AF.Identity
                    )
            return xT

        def qkv_stage(i):
            """LN1 + q/k/v projections for depth i (all PSUM via psmo)."""
            xT = layer_norm_t(f"1_{i}", early=(i == 0))
            qkT = [[None, None], [None, None]]
            for u in range(2):
                for t in range(2):
                    ps = psmo.tile([128, N], F32, tag="o")
                    nc.tensor.matmul(
                        ps, qkw_s[:, i, t, u, :], xT[:], start=True, stop=True
                    )
                    sb = wrk.tile([128, N], BF16, tag=f"qk{t}{u}")
                    nc.scalar.activation(
                        sb, ps, AF.Identity, bias=qkb_s[:, i, t, u : u + 1]
                    )
                    qkT[t][u] = sb
            # v computed k-major directly: out[k, (h,s)] per kt half
            for kt in range(2):
                vp2 = psmo.tile([128, 128], F32, tag="o")
                nc.tensor.matmul(
                    vp2, xT[:, kt * 128 : (kt + 1) * 128], vw_s[:, i, :],
                    start=True, stop=True,
                )
                if os.environ.get("VXACT", "1") == "1":
                    nc.scalar.activation(
                        vxt[kt][:, :, 0:16],
                        vp2.rearrange("p (h c) -> p h c", h=H), AF.Identity,
                    )
                else:
                    nc.vector.tensor_copy(
                        out=vxt[kt][:, :, 0:16],
                        in_=vp2.rearrange("p (h c) -> p h c", h=H),
                    )
                if not vbz:
                    nc.gpsimd.tensor_tensor(
                        out=vxt[kt][:, :, 0:16], in0=vxt[kt][:, :, 0:16],
                        in1=vbB_s[:, i, :, :], op=ALU.add,
                    )
            return xT, qkT

        # depth-0 LN1/qkv emitted ahead of the phase-1 work loop: its ops
        # interleave into phase-1 queue gaps instead of serializing after
        pre0 = qkv_stage(0) if PRE0 else None

        rsgs, aps = {}, {}
        for t in range(NGW + 2):
            if t < NGW:
                G, relq, j = work[t]
                p1_Y(G, j, relq)
                if t % 2 == 1:
                    rsgs[t - 1], rsgs[t] = p1_VP(G)
            if 1 <= t + 0 and t - 1 >= 0 and t - 1 < NGW:
                G, relq, j = work[t - 1]
                aps[t - 1] = p1_A2(j, relq)
            if t - 2 >= 0:
                G, relq, j = work[t - 2]
                p1_M(G, aps.pop(t - 2), rsgs.pop(t - 2))

        # ---------------- Phase 2: transformer depths ---------------------
        for i in range(DP_EMIT):
            xT, qkT = pre0 if (i == 0 and pre0 is not None) else qkv_stage(i)

            # one o-accumulator bank per q-half (psma is idle after phase 1)
            o2q = []
            for _qh in range(2):
                o2q_t = psma.tile([128, 8, 18], F32, tag="a")
                o2q.append(o2q_t)
            Eks = {}
            for p4 in range(4):
                for kt in range(2):
                    spk = psm.tile([128, 2, N], F32, tag="b")
                    ih0 = i * 8 + 2 * p4
                    if SAFE_SPK:
                        for j2 in range(2):
                            h = 2 * p4 + j2
                            u, j4 = divmod(h, 4)
                            hp = slice(32 * j4, 32 * j4 + HS)
                            ih = ih0 + j2
                            # R first: depends only on R_T, so it can run
                            # ahead of the qkT pipeline
                            nc.tensor.matmul(
                                spk[:, j2, :], idb_s[:], R_T[:, kt, :, ih],
                                start=True, stop=False,
                            )
                            nc.tensor.matmul(
                                spk[:, j2, :],
                                qkT[1][u][hp, kt * 128 : (kt + 1) * 128],
                                qkT[0][u][hp, :],
                                start=False, stop=True,
                                tile_position=(32 * j4, 0),
                            )
                    else:
                        nc.tensor.matmul(
                            spk, idb_s[:],
                            R_T[:, kt, :, ih0 : ih0 + 2].rearrange(
                                "p q i -> p i q"
                            ),
                            start=True, stop=False, skip_group_check=True,
                        )
                        for j2 in range(2):
                            h = 2 * p4 + j2
                            u, j4 = divmod(h, 4)
                            hp = slice(32 * j4, 32 * j4 + HS)
                            nc.tensor.matmul(
                                spk[:, j2, :],
                                qkT[1][u][hp, kt * 128 : (kt + 1) * 128],
                                qkT[0][u][hp, :],
                                start=False, stop=(j2 == 1),
                                tile_position=(32 * j4, 0),
                                skip_group_check=True,
                            )
                    plk = wrk4.tile([128, 2, N], BF16, tag="pl2")
                    Ekk = ekp.tile([128, 2, N], BF16, tag="Ek")
                    nc.vector.tensor_tensor(
                        out=plk, in0=spk, in1=connS2_s[:, :, kt, :],
                        op=ALU.mult,
                    )
                    nc.scalar.activation(Ekk, plk, AF.Exp)
                    Eks[kt] = Ekk
                # q-major o accumulation: each q-half lands in its own bank
                for j2 in range(2):
                    h = 2 * p4 + j2
                    for qh in range(2):
                        for kt in range(2):
                            nc.tensor.matmul(
                                o2q[qh][:, h, :],
                                Eks[kt][:, j2, qh * 128 : (qh + 1) * 128],
                                vxt[kt][:, h, :],
                                start=(kt == 0), stop=(kt == 1),
                            )
            # per-half softmax normalize (one reciprocal + one multiply each)
            for qh in range(2):
                dinv = st.tile([128, 8, 1], F32, tag="dinv")
                nc.vector.reciprocal(out=dinv[:, :, 0], in_=o2q[qh][:, :, 16])
                ogq = wrk4.tile([128, 8, 16], BF16, tag="ogq")
                ia, ib = bass.broadcast_tensor_aps(o2q[qh][:, :, 0:16], dinv)
                nc.vector.tensor_tensor(out=ogq, in0=ia, in1=ib, op=ALU.mult)
                tp = tslot()
                nc.tensor.transpose(
                    tp, ogq.rearrange("p h s -> p (h s)"), idb_s[:]
                )
                ogT = wrk4.tile([128, 128], BF16, tag="ogT")
                if qh == 0 or not ACT_COPY:
                    nc.vector.tensor_copy(out=ogT, in_=tp)
                else:
                    nc.scalar.activation(ogT, tp, AF.Identity)
                prq = psmo.tile([128, 128], F32, tag="o")
                nc.tensor.matmul(
                    prq, ogT, projW_s[:, i, :], start=True, stop=True,
                )
                nc.vector.tensor_tensor(
                    out=jf[qh], in0=jf[qh], in1=prq, op=ALU.add
                )
                if not pbz:
                    nc.gpsimd.tensor_tensor(
                        out=jf[qh], in0=jf[qh], in1=projbB_s[:, i, :],
                        op=ALU.add,
                    )

            xT3 = layer_norm_t(f"3_{i}")
            h1p = psmo.tile([128, N], F32, tag="o")
            for qh in range(2):
                q0 = slice(qh * 128, (qh + 1) * 128)
                nc.tensor.matmul(
                    h1p[:, q0], mw1_s[:, i, :], xT3[:, q0],
                    start=True, stop=True,
                )
            xg = wrk.tile([128, N], BF16, tag="xg")
            for qh in range(2):
                q0 = slice(qh * 128, (qh + 1) * 128)
                ee = nc.vector if (qh == 0 or os.environ.get("GPOOL", "1") == "1") else nc.gpsimd
                # gelu_tanh(x) with the final "x + x*tanh" add absorbed into
                # the h2 matmul: h2 = (xg @ mw2h) + (xg*tg @ mw2h).
                # xg and x2 both read h1p directly (parallel ACT ops); t1g
                # rides ACT too so x2->t1g has no cross-engine hop.
                nc.scalar.activation(
                    xg[:, q0], h1p[:, q0], AF.Identity, bias=mb1_s[:, i : i + 1]
                )
                x2 = wrk4.tile([128, 128], BF16, tag="x2")
                nc.scalar.activation(
                    x2, h1p[:, q0], AF.Square, bias=mb1_s[:, i : i + 1]
                )
                t1g = wrk4.tile([128, 128], BF16, tag="t1g")
                nc.scalar.activation(
                    t1g, x2, AF.Identity, bias=1.0, scale=GELC1
                )
                ug = wrk4.tile([128, 128], BF16, tag="ug")
                ee.tensor_tensor(out=ug, in0=xg[:, q0], in1=t1g, op=ALU.mult)
                tg = wrk4.tile([128, 128], BF16, tag="tg")
                nc.scalar.activation(tg, ug, AF.Tanh, scale=GELC0)
                xt2 = wrk4.tile([128, 128], BF16, tag="xt2")
                ee.tensor_tensor(out=xt2, in0=xg[:, q0], in1=tg, op=ALU.mult)
                h2t = psmo.tile([128, 128], F32, tag="o")
                nc.tensor.matmul(
                    h2t, xg[:, q0], mw2h_s[:, i, :], start=True, stop=False
                )
                nc.tensor.matmul(
                    h2t, xt2, mw2h_s[:, i, :], start=False, stop=True
                )
                nc.vector.tensor_tensor(
                    out=jf[qh], in0=jf[qh], in1=h2t, op=ALU.add
                )
                if not m2z:
                    nc.gpsimd.tensor_tensor(
                        out=jf[qh], in0=jf[qh], in1=mb2B_s[:, i, :],
                        op=ALU.add,
                    )

        # ---------------- decoder ----------------------------------------
        xTf = layer_norm_t("f")
        op_ps = psmo.tile([90, N], F32, tag="o")
        outT = wrk.tile([90, N], F32, tag="outT")
        of = wrk4.tile([128, 2, 90], F32, tag="of")
        for qt in range(2):
            q0 = slice(qt * 128, (qt + 1) * 128)
            nc.tensor.matmul(
                op_ps[:, q0], Wd_s[:], xTf[:, q0], start=True, stop=True
            )
            nc.scalar.activation(
                outT[:, q0], op_ps[:, q0], AF.Identity, bias=bd_s[:]
            )
            tp = psmo.tile([128, 90], F32, tag="o")
            nc.tensor.transpose(tp, outT[:, q0], idf_s[:90, :90])
            nc.scalar.activation(of[:, qt, :], tp, AF.Identity)
            nc.sync.dma_start(
                out=out_d.rearrange("(a p) c -> p a c", a=2)[:, qt, :],
                in_=of[:, qt, :],
            )

    nc.compile()
    return nc


def kernel(**inputs):
    global last_results
    w = _fold(inputs)
    rel = np.asarray(inputs["relation_in"], np.float32)
    conn = np.asarray(inputs["conn"], np.float32)
    joint = np.asarray(inputs["joint_in"], np.float32)

    br_zero = w.pop("_brz")
    m2z = w.pop("_m2z")
    pbz = w.pop("_pbz")
    vbz = w.pop("_vbz")
    Apre = w.pop("_Apre")
    Apost = w.pop("_Apost")
    Wj = w.pop("_Wj")
    in_maps = []
    for b in range(B):
        m = dict(w)
        flat = np.empty((27, NN), np.float32)
        flat[0:26] = rel[b].reshape(NN, 26).T
        flat[26] = 1.0
        fv = flat.reshape(27, 8, 16, 4, 128)  # (f, D, t2, c, p)
        rq = np.zeros((8, 4, 32, 16, 128), NPBF)  # (D, c, f32, t2, p)
        rq[:, :, 0:27] = fv.transpose(1, 3, 0, 2, 4).astype(NPBF)
        m["relq"] = np.ascontiguousarray(rq).reshape(8, 128, 2048)
        kk = conn[b].T.reshape(2, 128, N)  # (kt, k, q)
        connS2 = np.ascontiguousarray(
            np.broadcast_to(kk.transpose(1, 0, 2)[:, None, :, :], (128, 2, 2, N))
        ).astype(NPBF)
        m["blobA"] = np.concatenate(
            [Apre, connS2.reshape(128, -1), Apost], axis=1
        )
        m["blobC"] = np.concatenate(
            [Wj, np.ascontiguousarray(joint[b].T)], axis=1
        ).astype(np.float32)
        in_maps.append(m)

    nc = _build(br_zero=br_zero, m2z=m2z, pbz=pbz, vbz=vbz)
    last_results = run_bass_kernel_spmd(nc, in_maps, core_ids=list(range(B)))
    out = np.stack([r["out"] for r in last_results.results])
    return out.astype(np.float32)


# revision 57
# speedup vs baseline: 1.0057x; 1.0057x over previous
"""AsymFormer forward on 8 TRN2 NeuronCores — data-parallel over batch.

v4 design (from v3):
 - B=8 -> one batch element per core, no collectives.
 - Phase 1 (relation branch): same matmul structure, but the rsqrt
   Newton-refinement chain (6 gpsimd ops/group) is reduced to a single
   pow(x,-0.5) like the LN path; the var+eps PSUM->SBUF move runs
   entirely on ACT (keeps DVE a pure back-to-back p1_M stream) and is
   batched per PAIR of groups (adjacent var slots in vbig), as is the
   pow — the phase-1 cadence is max(ACT 612+106, DVE 658) per group.
 - Phase 2 attention restructured:
   * per-(head,kt) score groups ordered R-add first (depends only on
     R_T), then the q.k matmul accumulates on top.
   * o = E@v computed q-major (lhsT = E chunk): all 8 heads + softmax
     denominators land in one PSUM bank per q-half -> one reciprocal +
     one broadcast-multiply per half per depth (replaces 8 reciprocal +
     8 partition_broadcast + 8 multiplies).
   * proj applied with ogT as lhsT -> output lands token-major, residual
     add needs no transpose; v computed k-major directly (no vT
     transposes, no v-bias ACT op; generic-bias fallbacks ride blobF).
 - Depth-0 LN1/qkv is NOT hoisted ahead of phase 1 (PRE0=0): its ACT
   ops would inject into the cadence-critical phase-1 ACT stream. Its
   LN transposes still route via psmo (early=True) because the vbig
   bank carries phase-1 var traffic until just before depth 0.
 - MLP gelu: x2 squared straight from PSUM (parallel with xg); t1g on
   ACT right after x2 (no cross-engine hop); the final "x + x*tanh" add
   is absorbed into the h2 matmul (PE accumulates xg@W + xt2@W); all
   tensor_tensor hops on DVE (GPSIMD's higher per-op latency cost more
   in chain latency than it saved in DVE occupancy).
 - Hardware constraints learned: DMAs only from SP/ACT (HWDGE) queues;
   TensorScalar is NOT a valid Pool opcode; PSUM accumulation groups
   must close before another group's start=True touches the same bank.
"""

import os
import sys

sys.path.insert(0, "/opt/trn_rl_repo")

import numpy as np

import concourse.bacc as bacc
import concourse.bass as bass
import concourse.mybir as mybir
import concourse.tile as tile
from concourse.bass_utils import run_bass_kernel_spmd

B, N, C, H, DP = 8, 256, 128, 8, 4
HS = C // H  # 16
SCALE = 0.25
NN = N * N
F32 = mybir.dt.float32
BF16 = mybir.dt.bfloat16
F32R = mybir.dt.float32r
NPBF = mybir.dt.np(BF16)
AF = mybir.ActivationFunctionType
ALU = mybir.AluOpType
GELC0 = 0.7978845608028654
GELC1 = 0.044715

NG = 32          # phase-1 groups (2048 rel-rows each)
# blobA: qkw | vw | mw1 | mw2h | connS2 | Wd
BLOBA = DP * 2 * 2 * C + DP * C + DP * C + DP * C + 2 * 2 * N + 90
# blobB layout (f32, 128 partitions): qkb | vb | projb | mb1 | mb2 | bj | idf
BLOBB = DP * 2 * 2 + DP + DP + DP + DP + 1 + 128
# blobF: projW | projbB | vbB | mb2B  (128 partitions, bf16)
BLOBF = 4 * DP * C
DP_EMIT = int(os.environ.get("DP_EMIT", DP))
SAFE_SPK = os.environ.get("SAFE_SPK", "1") == "1"   # per-j2 score groups
ACT_COPY = os.environ.get("ACT_COPY", "0") == "1"   # ACT does half the PSUM->SBUF copies
PRE0 = os.environ.get("PRE0", "0") == "1"           # hoist depth-0 qkv ahead of phase 1

last_results = None


def _fold(inp):
    f = lambda k: np.asarray(inp[k], np.float32)
    w = {}
    # relation encoder collapse 26->128
    Wc = f("re_w1") @ f("re_w2") @ f("re_w3")
    bc = (f("re_b1") @ f("re_w2") + f("re_b2")) @ f("re_w3") + f("re_b3")
    P = np.eye(128, dtype=np.float64) - 1.0 / 128.0
    Mh = np.concatenate(
        [P @ Wc.T.astype(np.float64), P @ bc.astype(np.float64).reshape(128, 1)],
        axis=1,
    )
    G = Mh.T @ Mh
    Rc = np.linalg.cholesky(G + 1e-14 * np.eye(27)).T  # upper, Rc.T@Rc = G
    Rc = Rc.astype(np.float32)
    # block-diag yc matmul: (128, 128), [32c+f, 32c+r] = Rc[r, f]
    RcBD = np.zeros((128, 128), np.float32)
    for g in range(4):
        RcBD[32 * g : 32 * g + 27, 32 * g : 32 * g + 27] = Rc.T
    w["RcBD"] = RcBD.astype(NPBF)
    # o27sel (128, 4): [32g+r, g] = 1/128 for r < 27
    o27 = np.zeros((128, 4), np.float32)
    for g in range(4):
        o27[32 * g : 32 * g + 27, g] = 1.0 / 128.0
    w["o27sel"] = o27.astype(NPBF)
    # ln2+SCALE fold into rconv -> Wr2 (128, 32), mean-centered
    Wr = np.empty((128, DP, H), np.float32)
    br = np.empty((DP, H), np.float32)
    for i in range(DP):
        Wr[:, i, :] = SCALE * (f("ln2_g")[i][:, None] * f("rconv_w")[i])
        br[i] = SCALE * (f("ln2_b")[i] @ f("rconv_w")[i] + f("rconv_b")[i])
    Wr2 = Wr.reshape(128, DP * H)
    Wr2 = Wr2 - np.ones((128, 1), np.float32) * (Wr2.sum(0, keepdims=True) / 128.0)
    WqA = np.concatenate([Wc @ Wr2, (Wr2.T @ bc).reshape(1, 32)], axis=0)  # (27,32)
    WqBD = np.zeros((128, 128), np.float32)
    for c in range(4):
        WqBD[32 * c : 32 * c + 27, 32 * c : 32 * c + 32] = WqA
    w["WqA"] = WqBD.astype(NPBF)
    w["brO"] = np.ascontiguousarray(
        np.broadcast_to(br.reshape(-1), (128, 16, 32)).reshape(128, 512)
    ).astype(NPBF)
    # joint encoder collapse 96->128
    Wj = f("je_w1") @ f("je_w2") @ f("je_w3")
    bj = (f("je_b1") @ f("je_w2") + f("je_b2")) @ f("je_w3") + f("je_b3")
    w["Wj"] = np.ascontiguousarray(Wj)
    w["bj"] = np.ascontiguousarray(bj.reshape(128, 1))
    # per-depth: ln1 into qkv (+SCALE on q), ln3 into mw1
    qkvw = np.empty((DP, C, 3 * C), np.float32)
    qkvb = np.empty((DP, 3 * C), np.float32)
    mw1 = np.empty((DP, C, C), np.float32)
    mb1 = np.empty((DP, C), np.float32)
    for i in range(DP):
        qkvw[i] = f("ln1_g")[i][:, None] * f("qkv_w")[i]
        qkvb[i] = f("ln1_b")[i] @ f("qkv_w")[i] + f("qkv_b")[i]
        qkvw[i][:, :C] *= SCALE
        qkvb[i][:C] *= SCALE
        mw1[i] = f("ln3_g")[i][:, None] * f("mw1")[i]
        mb1[i] = f("ln3_b")[i] @ f("mw1")[i] + f("mb1")[i]
    # qk: heads 4u+j at 32j+s (s<16) within tile u
    qkw = np.zeros((C, DP, 2, 2, C), np.float32)
    qkb = np.zeros((C, DP, 2, 2), np.float32)
    for i in range(DP):
        for t in range(2):
            wt = qkvw[i][:, t * C : (t + 1) * C]
            bt = qkvb[i][t * C : (t + 1) * C]
            for h in range(H):
                u, j = divmod(h, 4)
                qkw[:, i, t, u, 32 * j : 32 * j + HS] = wt[:, h * HS : (h + 1) * HS]
                qkb[32 * j : 32 * j + HS, i, t, u] = bt[h * HS : (h + 1) * HS]
    w["qkw"] = qkw.astype(NPBF)
    w["qkb"] = np.ascontiguousarray(qkb)
    w["vw"] = np.ascontiguousarray(qkvw.transpose(1, 0, 2)[:, :, 2 * C :]).astype(NPBF)
    vb = qkvb[:, 2 * C :]  # (DP, C)
    w["vb"] = np.ascontiguousarray(vb.T)
    # proj token-major: rows are (h*HS+s) = proj_w rows directly
    w["projW"] = np.ascontiguousarray(
        f("proj_w").transpose(1, 0, 2)
    ).astype(NPBF)  # (C, DP, C)
    projb = f("proj_b")  # (DP, C)
    w["projb"] = np.ascontiguousarray(projb.T)
    w["mw1"] = np.ascontiguousarray(mw1.transpose(1, 0, 2)).astype(NPBF)
    w["mb1"] = np.ascontiguousarray(mb1.T)
    w["mw2h"] = np.ascontiguousarray(0.5 * f("mw2").transpose(1, 0, 2)).astype(NPBF)
    w["mb2"] = np.ascontiguousarray(f("mb2").T)
    # decoder with final LN affine folded
    Wdc = f("dw1") @ f("dw2") @ f("dw3")
    Wd = f("ng")[:, None] * Wdc
    bd = f("nb") @ Wdc + (f("db1") @ f("dw2") + f("db2")) @ f("dw3") + f("db3")
    w["Wd"] = np.ascontiguousarray(Wd).astype(NPBF)
    w["bd"] = np.ascontiguousarray(bd.reshape(90, 1))
    idb = np.eye(128, dtype=np.float32).astype(NPBF)
    idf = np.eye(128, dtype=np.float32)
    out = {}
    out["_Apre"] = np.concatenate([
        w["qkw"].reshape(128, -1), w["vw"].reshape(128, -1),
        w["mw1"].reshape(128, -1), w["mw2h"].reshape(128, -1),
    ], axis=1)
    out["_Apost"] = w["Wd"]
    out["blobI"] = np.concatenate([w["o27sel"], idb, w["brO"]], axis=1)
    out["blobB"] = np.concatenate([
        w["qkb"].reshape(128, -1), w["vb"], w["projb"], w["mb1"], w["mb2"],
        w["bj"], idf,
    ], axis=1).astype(np.float32)
    out["_Wj"] = w["Wj"]
    out["blobD"] = w["RcBD"]
    out["blobE"] = w["WqA"]
    # blobF: projW | projbB | vbB | mb2B (all broadcast along partitions
    # for the bias tiles; only read when the corresponding bias is nonzero)
    projbB = np.broadcast_to(projb.reshape(1, DP * C), (128, DP * C))
    vbB = np.broadcast_to(vb.reshape(1, DP * C), (128, DP * C))
    mb2B = np.broadcast_to(f("mb2").reshape(1, DP * C), (128, DP * C))
    out["blobF"] = np.ascontiguousarray(np.concatenate([
        w["projW"].reshape(128, -1).astype(np.float32),
        projbB, vbB, mb2B,
    ], axis=1)).astype(NPBF)
    out["blobH"] = w["bd"]
    out["_brz"] = bool(np.all(w["brO"] == 0))
    out["_m2z"] = bool(np.all(w["mb2"] == 0))
    out["_pbz"] = bool(np.all(projb == 0))
    out["_vbz"] = bool(np.all(vb == 0))
    return out


def _build(br_zero=True, m2z=True, pbz=True, vbz=True):
    nc = bacc.Bacc(None, target_bir_lowering=False)

    def din(name, shape, dt=F32):
        return nc.dram_tensor(name, list(shape), dt, kind="ExternalInput")

    relq_d = din("relq", (8, 128, 2048), BF16)
    # const blobs (concatenated along free dim, per partition-count/dtype)
    blobA_d = din("blobA", (128, BLOBA), BF16)   # 128-part bf16 weights
    blobB_d = din("blobB", (128, BLOBB))         # 128-part f32 biases/idf
    blobC_d = din("blobC", (96, 128 + N), F32R)  # Wj | jT
    blobD_d = din("blobD", (128, 128), BF16)     # RcBD
    blobI_d = din("blobI", (128, 132 + 512), BF16)  # o27sel | idb | brO
    blobE_d = din("blobE", (128, 128), BF16)     # WqBD block-diag
    blobF_d = din("blobF", (128, BLOBF), BF16)   # projW | projbB | vbB | mb2B
    blobH_d = din("blobH", (90, 1))              # bd
    out_d = nc.dram_tensor("out", [N, 90], F32, kind="ExternalOutput")

    from contextlib import ExitStack

    with tile.TileContext(nc) as tc, ExitStack() as ctx, nc.allow_low_precision(
        reason="bf16 pipeline; end-to-end precision checked in test"
    ):
        const = ctx.enter_context(tc.tile_pool(name="const", bufs=1))
        zin = ctx.enter_context(tc.tile_pool(name="zin", bufs=8))
        st = ctx.enter_context(tc.tile_pool(name="st", bufs=8))
        wrk = ctx.enter_context(tc.tile_pool(name="wrk", bufs=4))
        wrk4 = ctx.enter_context(tc.tile_pool(name="wrk4", bufs=4))
        ekp = ctx.enter_context(tc.tile_pool(name="ekp", bufs=6))
        psm = ctx.enter_context(tc.tile_pool(name="psm", bufs=int(os.environ.get("PSMB","3")), space="PSUM"))
        psma = ctx.enter_context(tc.tile_pool(name="psma", bufs=2, space="PSUM"))
        psmv = ctx.enter_context(tc.tile_pool(name="psmv", bufs=1, space="PSUM"))
        psmo = ctx.enter_context(tc.tile_pool(name="psmo", bufs=int(os.environ.get("PSMOB","2")), space="PSUM"))

        def cload(dt_handle, shape, tag, dt=F32, eng=None):
            t = const.tile(list(shape), dt, tag=tag)
            (eng or nc.scalar).dma_start(out=t, in_=dt_handle[:])
            return t

        RcBD_s = cload(blobD_d, (128, 128), "RcBD", BF16)
        WqA_s = cload(blobE_d, (128, 128), "WqA", BF16)
        mhB_s = const.tile([128, 4, 4], F32, tag="mhB")
        nc.vector.memset(mhB_s[:], -0.5)
        eps_s = const.tile([128, 1], F32, tag="eps")
        nc.vector.memset(eps_s[:], 1e-5)
        mh1_s = const.tile([128, 1], F32, tag="mh1")
        nc.vector.memset(mh1_s[:], -0.5)
        mh2_s = const.tile([128, 2, 1], F32, tag="mh2")
        nc.vector.memset(mh2_s[:], -0.5)
        mhP_s = const.tile([128, 2, 16], F32, tag="mhP")
        nc.vector.memset(mhP_s[:], -0.5)
        blobI_s = cload(blobI_d, (128, 132 + 512), "blobI", BF16)
        o27_s = blobI_s[:, 0:4]
        idb_s = blobI_s[:, 4:132]
        brO_s = blobI_s[:, 132:644].rearrange("p (kt q h) -> p kt q h", kt=2, q=8)
        blobC_s = cload(blobC_d, (96, 128 + N), "blobC", F32R)
        Wj_s = blobC_s[:, 0:128]
        jT_s = blobC_s[:, 128 : 128 + N]
        blobB_s = cload(blobB_d, (128, BLOBB), "blobB")
        blobA_s = const.tile([128, BLOBA], BF16, tag="blobA")
        _o = [0]

        def _cutA(n, shape=None):
            v = blobA_s[:, _o[0] : _o[0] + n]
            _o[0] += n
            return v

        qkw_s = _cutA(DP * 2 * 2 * C).rearrange(
            "p (i t u c) -> p i t u c", i=DP, t=2, u=2
        )
        vw_s = _cutA(DP * C).rearrange("p (i c) -> p i c", i=DP)
        mw1_s = _cutA(DP * C).rearrange("p (i c) -> p i c", i=DP)
        mw2h_s = _cutA(DP * C).rearrange("p (i c) -> p i c", i=DP)
        connS2_s = _cutA(2 * 2 * N).rearrange("p (a kt q) -> p a kt q", a=2, kt=2)
        Wd_s = _cutA(90)
        _ob = [0]

        def _cutB(n):
            v = blobB_s[:, _ob[0] : _ob[0] + n]
            _ob[0] += n
            return v

        qkb_s = _cutB(DP * 2 * 2).rearrange("p (i t u) -> p i t u", i=DP, t=2)
        vb_s = _cutB(DP)
        projb_s = _cutB(DP)
        mb1_s = _cutB(DP)
        mb2_s = _cutB(DP)
        bj_s = _cutB(1)
        idf_s = _cutB(128)

        R_T = const.tile([128, 2, N, 32], BF16, tag="R_T")
        vbig = psmv.tile([128, 512], F32, tag="vt")  # shared bank: var slots + transpose slots
        tbig = vbig[:, 128:512].bitcast(BF16)  # (128, 768) bf16 -> 6 transpose slots
        tctr = [0]

        def tslot():
            r = tctr[0] % 6
            tctr[0] += 1
            return tbig[:, 128 * r : 128 * (r + 1)]

        # v_ext tiles (128, H, 18): cols 0-15 = v (per depth), col 16 = ones
        # (softmax denominator rides the same matmul), col 17 = pad
        vxt = []
        for kt in range(2):
            t = const.tile([128, H, 18], BF16, tag=f"vx{kt}")
            nc.vector.memset(t[:, :, 16:18], 0.0)
            nc.vector.memset(t[:, :, 16:17], 1.0)
            vxt.append(t)

        # ---------------- Phase 1: relation branch -> R_T ----------------
        def p1_Y(G, j, relq):
            yc_ps = psm.tile([128, 512], F32, tag="b")
            nc.tensor.matmul(
                yc_ps, RcBD_s[:], relq[:, j * 512 : (j + 1) * 512],
                start=True, stop=True,
            )
            ycsq = wrk.tile([128, 512], BF16, tag="ycsq")
            nc.scalar.activation(ycsq, yc_ps, AF.Square)
            var_q = vbig[:, 16 * (G % 8) : 16 * (G % 8) + 16].rearrange(
                "p (t c) -> p t c", t=4
            )
            for s in range(4):
                nc.tensor.matmul(
                    var_q[:, s, :],
                    ycsq[:, s * 128 : (s + 1) * 128],
                    o27_s[:], start=True, stop=True,
                )
            # veps/pow are emitted per PAIR of groups in p1_VP below

        def p1_VP(G):
            """veps + pow for groups (G-1, G), G odd: their var slots are
            adjacent in vbig, so one ACT op (PSUM->SBUF + eps) and one Pool
            pow cover both — halves the per-group ACT/Pool tax on the
            phase-1 cadence (ACT is the cadence-critical engine)."""
            s0 = 16 * ((G - 1) % 8)
            vpair = vbig[:, s0 : s0 + 32]
            veps = st.tile([128, 2, 16], F32, tag="veps")
            nc.scalar.activation(veps, vpair.rearrange(
                "p (g a) -> p g a", g=2), AF.Identity, bias=eps_s[:])
            rsg = st.tile([128, 2, 16, 1], F32, tag="rsg")
            nc.gpsimd.tensor_tensor(
                out=rsg[:, :, :, 0], in0=veps, in1=mhP_s, op=ALU.pow,
            )
            return (
                rsg[:, 0].rearrange("p (t cc kt) one -> p t cc kt one",
                                    t=4, cc=2),
                rsg[:, 1].rearrange("p (t cc kt) one -> p t cc kt one",
                                    t=4, cc=2),
            )

        def p1_A2(j, relq):
            a_ps = psma.tile([128, 4, 4, 32], F32, tag="a")
            for t in range(4):
                t2 = 4 * j + t
                nc.tensor.matmul(
                    a_ps[:, t, :, :],
                    relq[:, t2 * 128 : (t2 + 1) * 128],
                    WqA_s[:], start=True, stop=True,
                )
            return a_ps

        def p1_M(G, a_ps, rsg):
            out = R_T[:, :, 8 * G : 8 * G + 8, :].rearrange(
                "p kt (t cc) h -> p t cc kt h", t=4
            )
            av = a_ps.rearrange("p t (cc kt) h -> p t cc kt h", cc=2)
            ia, ib = bass.broadcast_tensor_aps(av, rsg)
            nc.vector.tensor_tensor(out=out, in0=ia, in1=ib, op=ALU.mult)
            if not br_zero:
                sl = R_T[:, :, 8 * G : 8 * G + 8, :]
                eng = nc.vector if G % 2 == 0 else nc.gpsimd
                eng.tensor_tensor(out=sl, in0=sl, in1=brO_s[:], op=ALU.add)

        work = []  # (G, relq, j)
        relq_split = os.environ.get("RELQ_SPLIT", "0") == "1"
        for D in range(8):
            relq = zin.tile([128, 2048], BF16, tag="relq")
            eng = nc.scalar if (relq_split and D % 2 == 1) else nc.sync
            eng.dma_start(out=relq, in_=relq_d[D])
            for j in range(4):
                work.append((4 * D + j, relq, j))
        NGW = len(work)
        # big phase-2 blob: issued on the sync queue (after the rel DMAs) so
        # the ACT queue is never blocked behind a long transfer
        nc.sync.dma_start(out=blobA_s, in_=blobA_d[:])
        blobF_s = const.tile([128, BLOBF], BF16, tag="blobF")
        nc.sync.dma_start(out=blobF_s, in_=blobF_d[:])
        projW_s = blobF_s[:, 0 : DP * C].rearrange("p (i c) -> p i c", i=DP)
        projbB_s = blobF_s[:, DP * C : 2 * DP * C].rearrange(
            "p (i c) -> p i c", i=DP
        )
        vbB_s = blobF_s[:, 2 * DP * C : 3 * DP * C].rearrange(
            "p (i h s) -> p i h s", i=DP, h=H
        )
        mb2B_s = blobF_s[:, 3 * DP * C : 4 * DP * C].rearrange(
            "p (i c) -> p i c", i=DP
        )
        bd_s = const.tile([90, 1], F32, tag="bd")
        nc.sync.dma_start(out=bd_s, in_=blobH_d[:])

        # ---------------- joint encoder -> jf (token-major) --------------
        # jf lives in ONE tile (128, 2, 128): both token-halves share a
        # single bn_stats / veps / pow per LayerNorm (bn_stats computes
        # per-segment stats along the middle dim)
        jf_all = const.tile([128, 2, 128], F32, tag="jfA")
        jf = [jf_all[:, 0, :], jf_all[:, 1, :]]
        jp = psmo.tile([128, N], F32, tag="o")
        nc.tensor.matmul(jp, Wj_s[:], jT_s[:], start=True, stop=True)
        jfT = wrk.tile([128, N], F32, tag="jfT")
        nc.scalar.activation(jfT, jp, AF.Identity, bias=bj_s[:])
        for qt in range(2):
            tp = psmo.tile([128, 128], F32, tag="o")
            nc.tensor.transpose(tp, jfT[:, qt * 128 : (qt + 1) * 128], idf_s[:])
            nc.vector.tensor_copy(out=jf[qt], in_=tp[:])

        def layer_norm_t(tag, early=False):
            """token-major standardize -> feature-major (128, 256) bf16.

            early=True routes the transpose through psmo (phase 1 owns the
            vbig bank that tslot() lives in — a PE write there while phase-1
            DVE/ACT read var slots would be a PSUM bank collision).
            """
            xT = wrk.tile([128, N], BF16, tag=f"xT{tag}")
            mvs, rss, xhs = [], [], []
            for qt in range(2):
                st6 = st.tile([128, 6], F32, tag="st6")
                nc.vector.bn_stats(out=st6, in_=jf[qt])
                mv = st.tile([128, 2], F32, tag="mv")
                nc.vector.bn_aggr(out=mv, in_=st6[:])
                veps = st.tile([128, 1], F32, tag="veps1")
                nc.vector.tensor_scalar(
                    out=veps, in0=mv[:, 1:2], scalar1=1e-5, scalar2=None,
                    op0=ALU.add,
                )
                rs0 = st.tile([128, 1], F32, tag="rs0")
                # gpsimd supports tensor_tensor only (TensorScalar fails the
                # hardware engine check on Pool)
                nc.gpsimd.tensor_tensor(
                    out=rs0, in0=veps, in1=mh1_s[:], op=ALU.pow
                )
                mvs.append(mv)
                rss.append(rs0)
            for qt in range(2):
                xh = wrk4.tile([128, 128], BF16, tag="xh")
                nc.vector.tensor_scalar(
                    out=xh, in0=jf[qt], scalar1=mvs[qt][:, 0:1],
                    scalar2=rss[qt][:],
                    op0=ALU.subtract, op1=ALU.mult,
                )
                xhs.append(xh)
            for qt in range(2):
                if early:
                    tp = psmo.tile([128, 128], BF16, tag="o")
                else:
                    tp = tslot()
                nc.tensor.transpose(tp, xhs[qt][:], idb_s[:])
                if qt == 0 or not ACT_COPY:
                    nc.vector.tensor_copy(
                        out=xT[:, qt * 128 : (qt + 1) * 128], in_=tp
                    )
                else:
                    nc.scalar.activation(
                        xT[:, qt * 128 : (qt + 1) * 128], tp, AF.Identity
                    )
            return xT

        def qkv_stage(i):
            """LN1 + q/k/v projections for depth i (all PSUM via psmo)."""
            xT = layer_norm_t(f"1_{i}", early=(i == 0))
            qkT = [[None, None], [None, None]]
            for u in range(2):
                for t in range(2):
                    ps = psmo.tile([128, N], F32, tag="o")
                    nc.tensor.matmul(
                        ps, qkw_s[:, i, t, u, :], xT[:], start=True, stop=True
                    )
                    sb = wrk.tile([128, N], BF16, tag=f"qk{t}{u}")
                    nc.scalar.activation(
                        sb, ps, AF.Identity, bias=qkb_s[:, i, t, u : u + 1]
                    )
                    qkT[t][u] = sb
            # v computed k-major directly: out[k, (h,s)] per kt half
            for kt in range(2):
                vp2 = psmo.tile([128, 128], F32, tag="o")
                nc.tensor.matmul(
                    vp2, xT[:, kt * 128 : (kt + 1) * 128], vw_s[:, i, :],
                    start=True, stop=True,
                )
                if os.environ.get("VXACT", "1") == "1":
                    nc.scalar.activation(
                        vxt[kt][:, :, 0:16],
                        vp2.rearrange("p (h c) -> p h c", h=H), AF.Identity,
                    )
                else:
                    nc.vector.tensor_copy(
                        out=vxt[kt][:, :, 0:16],
                        in_=vp2.rearrange("p (h c) -> p h c", h=H),
                    )
                if not vbz:
                    nc.gpsimd.tensor_tensor(
                        out=vxt[kt][:, :, 0:16], in0=vxt[kt][:, :, 0:16],
                        in1=vbB_s[:, i, :, :], op=ALU.add,
                    )
            return xT, qkT

        # depth-0 LN1/qkv emitted ahead of the phase-1 work loop: its ops
        # interleave into phase-1 queue gaps instead of serializing after
        pre0 = qkv_stage(0) if PRE0 else None

        rsgs, aps = {}, {}
        for t in range(NGW + 2):
            if t < NGW:
                G, relq, j = work[t]
                p1_Y(G, j, relq)
                if t % 2 == 1:
                    rsgs[t - 1], rsgs[t] = p1_VP(G)
            if 1 <= t + 0 and t - 1 >= 0 and t - 1 < NGW:
                G, relq, j = work[t - 1]
                aps[t - 1] = p1_A2(j, relq)
            if t - 2 >= 0:
                G, relq, j = work[t - 2]
                p1_M(G, aps.pop(t - 2), rsgs.pop(t - 2))

        # ---------------- Phase 2: transformer depths ---------------------
        for i in range(DP_EMIT):
            xT, qkT = pre0 if (i == 0 and pre0 is not None) else qkv_stage(i)

            # one o-accumulator bank per q-half (psma is idle after phase 1)
            o2q = []
            for _qh in range(2):
                o2q_t = psma.tile([128, 8, 18], F32, tag="a")
                o2q.append(o2q_t)
            Eks = {}
            for p4 in range(4):
                for kt in range(2):
                    spk = psm.tile([128, 2, N], F32, tag="b")
                    ih0 = i * 8 + 2 * p4
                    if SAFE_SPK:
                        for j2 in range(2):
                            h = 2 * p4 + j2
                            u, j4 = divmod(h, 4)
                            hp = slice(32 * j4, 32 * j4 + HS)
                            ih = ih0 + j2
                            # R first: depends only on R_T, so it can run
                            # ahead of the qkT pipeline
                            nc.tensor.matmul(
                                spk[:, j2, :], idb_s[:], R_T[:, kt, :, ih],
                                start=True, stop=False,
                            )
                            nc.tensor.matmul(
                                spk[:, j2, :],
                                qkT[1][u][hp, kt * 128 : (kt + 1) * 128],
                                qkT[0][u][hp, :],
                                start=False, stop=True,
                                tile_position=(32 * j4, 0),
                            )
                    else:
                        nc.tensor.matmul(
                            spk, idb_s[:],
                            R_T[:, kt, :, ih0 : ih0 + 2].rearrange(
                                "p q i -> p i q"
                            ),
                            start=True, stop=False, skip_group_check=True,
                        )
                        for j2 in range(2):
                            h = 2 * p4 + j2
                            u, j4 = divmod(h, 4)
                            hp = slice(32 * j4, 32 * j4 + HS)
                            nc.tensor.matmul(
                                spk[:, j2, :],
                                qkT[1][u][hp, kt * 128 : (kt + 1) * 128],
                                qkT[0][u][hp, :],
                                start=False, stop=(j2 == 1),
                                tile_position=(32 * j4, 0),
                                skip_group_check=True,
                            )
                    plk = wrk4.tile([128, 2, N], BF16, tag="pl2")
                    Ekk = ekp.tile([128, 2, N], BF16, tag="Ek")
                    nc.vector.tensor_tensor(
                        out=plk, in0=spk, in1=connS2_s[:, :, kt, :],
                        op=ALU.mult,
                    )
                    nc.scalar.activation(Ekk, plk, AF.Exp)
                    Eks[kt] = Ekk
                # q-major o accumulation: each q-half lands in its own bank
                for j2 in range(2):
                    h = 2 * p4 + j2
                    for qh in range(2):
                        for kt in range(2):
                            nc.tensor.matmul(
                                o2q[qh][:, h, :],
                                Eks[kt][:, j2, qh * 128 : (qh + 1) * 128],
                                vxt[kt][:, h, :],
                                start=(kt == 0), stop=(kt == 1),
                            )
            # per-half softmax normalize, emitted stage-major so same-engine
            # ops are queue-adjacent and one half's blocked op never parks
            # ahead of the other half's ready op
            ogqs, ogTs, prqs = [], [], []
            for qh in range(2):
                dinv = st.tile([128, 8, 1], F32, tag="dinv")
                nc.vector.reciprocal(out=dinv[:, :, 0], in_=o2q[qh][:, :, 16])
                ogq = wrk4.tile([128, 8, 16], BF16, tag="ogq")
                ia, ib = bass.broadcast_tensor_aps(o2q[qh][:, :, 0:16], dinv)
                nc.vector.tensor_tensor(out=ogq, in0=ia, in1=ib, op=ALU.mult)
                ogqs.append(ogq)
            for qh in range(2):
                tp = tslot()
                nc.tensor.transpose(
                    tp, ogqs[qh].rearrange("p h s -> p (h s)"), idb_s[:]
                )
                ogT = wrk4.tile([128, 128], BF16, tag="ogT")
                if qh == 0 or not ACT_COPY:
                    nc.vector.tensor_copy(out=ogT, in_=tp)
                else:
                    nc.scalar.activation(ogT, tp, AF.Identity)
                ogTs.append(ogT)
            for qh in range(2):
                prq = psmo.tile([128, 128], F32, tag="o")
                nc.tensor.matmul(
                    prq, ogTs[qh], projW_s[:, i, :], start=True, stop=True,
                )
                prqs.append(prq)
            for qh in range(2):
                nc.vector.tensor_tensor(
                    out=jf[qh], in0=jf[qh], in1=prqs[qh], op=ALU.add
                )
                if not pbz:
                    nc.gpsimd.tensor_tensor(
                        out=jf[qh], in0=jf[qh], in1=projbB_s[:, i, :],
                        op=ALU.add,
                    )

            xT3 = layer_norm_t(f"3_{i}")
            h1p = psmo.tile([128, N], F32, tag="o")
            for qh in range(2):
                q0 = slice(qh * 128, (qh + 1) * 128)
                nc.tensor.matmul(
                    h1p[:, q0], mw1_s[:, i, :], xT3[:, q0],
                    start=True, stop=True,
                )
            # gelu_tanh(x) with the final "x + x*tanh" add absorbed into the
            # h2 matmul: h2 = (xg @ mw2h) + (xg*tg @ mw2h). xg and x2 both
            # read h1p directly (parallel ACT ops); t1g rides ACT too so
            # x2->t1g has no cross-engine hop. Stage-major across halves for
            # same-engine queue adjacency.
            xg = wrk.tile([128, N], BF16, tag="xg")
            t1gs, ugs, tgs, xt2s, h2ts = [], [], [], [], []
            for qh in range(2):
                q0 = slice(qh * 128, (qh + 1) * 128)
                nc.scalar.activation(
                    xg[:, q0], h1p[:, q0], AF.Identity, bias=mb1_s[:, i : i + 1]
                )
                x2 = wrk4.tile([128, 128], BF16, tag="x2")
                nc.scalar.activation(
                    x2, h1p[:, q0], AF.Square, bias=mb1_s[:, i : i + 1]
                )
                t1g = wrk4.tile([128, 128], BF16, tag="t1g")
                nc.scalar.activation(
                    t1g, x2, AF.Identity, bias=1.0, scale=GELC1
                )
                t1gs.append(t1g)
            for qh in range(2):
                q0 = slice(qh * 128, (qh + 1) * 128)
                ug = wrk4.tile([128, 128], BF16, tag="ug")
                nc.vector.tensor_tensor(
                    out=ug, in0=xg[:, q0], in1=t1gs[qh], op=ALU.mult
                )
                ugs.append(ug)
            for qh in range(2):
                tg = wrk4.tile([128, 128], BF16, tag="tg")
                nc.scalar.activation(tg, ugs[qh], AF.Tanh, scale=GELC0)
                tgs.append(tg)
            for qh in range(2):
                q0 = slice(qh * 128, (qh + 1) * 128)
                xt2 = wrk4.tile([128, 128], BF16, tag="xt2")
                nc.vector.tensor_tensor(
                    out=xt2, in0=xg[:, q0], in1=tgs[qh], op=ALU.mult
                )
                xt2s.append(xt2)
            for qh in range(2):
                q0 = slice(qh * 128, (qh + 1) * 128)
                h2t = psmo.tile([128, 128], F32, tag="o")
                nc.tensor.matmul(
                    h2t, xg[:, q0], mw2h_s[:, i, :], start=True, stop=False
                )
                nc.tensor.matmul(
                    h2t, xt2s[qh], mw2h_s[:, i, :], start=False, stop=True
                )
                h2ts.append(h2t)
            for qh in range(2):
                nc.vector.tensor_tensor(
                    out=jf[qh], in0=jf[qh], in1=h2ts[qh], op=ALU.add
                )
                if not m2z:
                    nc.gpsimd.tensor_tensor(
                        out=jf[qh], in0=jf[qh], in1=mb2B_s[:, i, :],
                        op=ALU.add,
                    )

        # ---------------- decoder ----------------------------------------
        xTf = layer_norm_t("f")
        op_ps = psmo.tile([90, N], F32, tag="o")
        outT = wrk.tile([90, N], F32, tag="outT")
        of = wrk4.tile([128, 2, 90], F32, tag="of")
        for qt in range(2):
            q0 = slice(qt * 128, (qt + 1) * 128)
            nc.tensor.matmul(
                op_ps[:, q0], Wd_s[:], xTf[:, q0], start=True, stop=True
            )
            nc.scalar.activation(
                outT[:, q0], op_ps[:, q0], AF.Identity, bias=bd_s[:]
            )
            tp = psmo.tile([128, 90], F32, tag="o")
            nc.tensor.transpose(tp, outT[:, q0], idf_s[:90, :90])
            nc.scalar.activation(of[:, qt, :], tp, AF.Identity)
            nc.sync.dma_start(
                out=out_d.rearrange("(a p) c -> p a c", a=2)[:, qt, :],
                in_=of[:, qt, :],
            )

    nc.compile()
    return nc


def kernel(**inputs):
    global last_results
    w = _fold(inputs)
    rel = np.asarray(inputs["relation_in"], np.float32)
    conn = np.asarray(inputs["conn"], np.float32)
    joint = np.asarray(inputs["joint_in"], np.float32)

    br_zero = w.pop("_brz")
    m2z = w.pop("_m2z")
    pbz = w.pop("_pbz")
    vbz = w.pop("_vbz")
    Apre = w.pop("_Apre")
    Apost = w.pop("_Apost")
    Wj = w.pop("_Wj")
    in_maps = []
    for b in range(B):
        m = dict(w)
        flat = np.empty((27, NN), np.float32)
        flat[0:26] = rel[b].reshape(NN, 26).T
        flat[26] = 1.0
        fv = flat.reshape(27, 8, 16, 4, 128)  # (f, D, t2, c, p)
        rq = np.zeros((8, 4, 32, 16, 128), NPBF)  # (D, c, f32, t2, p)
        rq[:, :, 0:27] = fv.transpose(1, 3, 0, 2, 4).astype(NPBF)
        m["relq"] = np.ascontiguousarray(rq).reshape(8, 128, 2048)
        kk = conn[b].T.reshape(2, 128, N)  # (kt, k, q)
        connS2 = np.ascontiguousarray(
            np.broadcast_to(kk.transpose(1, 0, 2)[:, None, :, :], (128, 2, 2, N))
        ).astype(NPBF)
        m["blobA"] = np.concatenate(
            [Apre, connS2.reshape(128, -1), Apost], axis=1
        )
        m["blobC"] = np.concatenate(
            [Wj, np.ascontiguousarray(joint[b].T)], axis=1
        ).astype(np.float32)
        in_maps.append(m)

    nc = _build(br_zero=br_zero, m2z=m2z, pbz=pbz, vbz=vbz)
    last_results = run_bass_kernel_spmd(nc, in_maps, core_ids=list(range(B)))
    out = np.stack([r["out"] for r in last_results.results])
    return out.astype(np.float32)
